# revision 60
# baseline (speedup 1.0000x reference)
"""Bidirectional attention block (RMSNorm -> QKV+RoPE -> SDPA -> out-proj -> residual)
on 8 Trainium2 NeuronCores.

Sharding: tensor-parallel over heads (2 heads/core) through attention, then a
per-batch AllToAll switches to token-parallel (2x256 tokens/core) for the output
projection + residual. Host only slices/concatenates numpy arrays.

Layout choices (vs the obvious mapping):
  - V is projected directly token-major (lhsT=x-chunk, rhs=w_v) so no transpose
    is ever needed for the attention AV matmul.
  - AV matmul emits [q, head_dim(+1)] (65-wide output) so the contraction over
    k-tiles pays 65 columns/step instead of 512, and the softmax denominator
    lands as a per-partition scalar (cheap normalize, no broadcast DMA).
  - RoPE rotate-half is a PE matmul with a constant +-1 permutation matrix.
  - rms(token) row broadcast is a PE rank-1 outer product, not a DRAM roundtrip.
  - rsqrt is computed as exp(-0.5*ln(s)) so the WHOLE kernel needs only the
    natural_log_exp activation table set (no exp<->sqrt table thrashing).
  - batch-1 sumsq runs on DVE (scalar_tensor_tensor accumulate over token-major
    x tiles) inside batch-0's attention sweep, off the PE critical path.

Shapes hardcoded for B=2, T=2048, D_MODEL=1024, N_HEADS=16, HEAD_DIM=64.
"""

import numpy as np
import ml_dtypes

import concourse.bass as bass
import concourse.tile as tile
from concourse import bacc, mybir
from concourse.alu_op_type import AluOpType
from concourse.bass_utils import run_bass_kernel_spmd

B, T, D = 2, 2048, 1024
H, HD = 16, 64
BT = B * T                      # 4096 tokens
N_CORES = 8
HPC = H // N_CORES              # 2 heads per core
JC = 3 * HPC * HD               # 384 qkv features per core
TPC = BT // N_CORES             # 512 tokens per core for out-proj (256/batch)
RMS_EPS = 1e-5
ROPE_BASE = 10000.0

BF = mybir.dt.bfloat16
F32 = mybir.dt.float32

QG = 512                        # q-span per attention group
NQG = T // QG                   # 4 q-groups per batch
NKT = T // 128                  # 16 k-tiles per batch
NTT = BT // 128                 # 32 token tiles globally


def build_kernel(nc, with_collective=True):
    # xT pre-chunked host-side: [8 (b ck), 128 d, 8c*512t] so each chunk is one
    # contiguous DMA; wq is jt-major [128, (jt c 128)]; wo/xres pre-arranged
    xT_ap = nc.dram_tensor("xT", [8, 128, 8 * 512], BF, kind="ExternalInput").ap()
    xrms_ap = nc.dram_tensor("xrms", [NTT, 128, D], BF, kind="ExternalInput").ap()
    xres_ap = nc.dram_tensor("xres", [128, 4 * D], F32, kind="ExternalInput").ap()
    wq_ap = nc.dram_tensor("wq", [128, 8 * JC], BF, kind="ExternalInput").ap()
    wo_ap = nc.dram_tensor("wo", [128, 8 * D], BF, kind="ExternalInput").ap()
    cos_ap = nc.dram_tensor("cosb", [128, T], BF, kind="ExternalInput").ap()
    sin_ap = nc.dram_tensor("sinb", [128, T], BF, kind="ExternalInput").ap()
    ropep_ap = nc.dram_tensor("ropep", [128, 128], BF, kind="ExternalInput").ap()
    y_ap = nc.dram_tensor("y", [TPC, D], F32, kind="ExternalOutput").ap()

    with tile.TileContext(nc) as tc:
        _body(nc, tc, dict(
            xT=xT_ap, xrms=xrms_ap, xres=xres_ap, wq=wq_ap, wo=wo_ap,
            cos=cos_ap, sin=sin_ap, ropep=ropep_ap, y=y_ap,
        ), with_collective)
    return nc


def _body(nc, tc, io, with_collective):
    from contextlib import ExitStack
    ctx = ExitStack()
    with ctx:
        singles = ctx.enter_context(tc.tile_pool(name="singles", bufs=1))
        xtp = ctx.enter_context(tc.tile_pool(name="xtp", bufs=2))
        work = ctx.enter_context(tc.tile_pool(name="work", bufs=2))
        exp_pool = ctx.enter_context(tc.tile_pool(name="exp", bufs=8))
        epi_pool = ctx.enter_context(tc.tile_pool(name="epi", bufs=2))
        out_sb = ctx.enter_context(tc.tile_pool(name="out_sb", bufs=2))
        dram = ctx.enter_context(tc.tile_pool(name="dram", bufs=1, space="DRAM"))
        stg_ctx = ctx.enter_context(ExitStack())
        stg_ps = stg_ctx.enter_context(
            tc.tile_pool(name="stg_ps", bufs=2, space="PSUM", side="right"))

        # ---- persistent constants / weights ----
        wq_sb = singles.tile([128, 8 * JC], BF)
        cos_sb = singles.tile([128, T], BF)
        sin_sb = singles.tile([128, T], BF)
        ropep_sb = singles.tile([128, 128], BF)

        def wq_dma():
            # contiguous host layout [p, (jt c j)]: q+k part first (gates the
            # first matmuls), v part second; both on the sync queue
            nc.sync.dma_start(wq_sb[:, 0:2048], io["wq"][:, 0:2048])
            nc.sync.dma_start(wq_sb[:, 2048:3072], io["wq"][:, 2048:3072])

        def trig_dma():
            nc.scalar.dma_start(cos_sb[:], io["cos"][:])
            nc.scalar.dma_start(sin_sb[:], io["sin"][:])
            nc.scalar.dma_start(ropep_sb[:], io["ropep"][:])

        ones_col = singles.tile([1, 128], BF)
        nc.vector.memset(ones_col[:], 1.0)

        ones128 = singles.tile([128, 1], BF)
        nc.vector.memset(ones128[:], 1.0)
        eps_row = singles.tile([1, 1], F32)
        nc.vector.memset(eps_row[:], RMS_EPS)
        eps_col = singles.tile([128, 1], F32)
        nc.vector.memset(eps_col[:], RMS_EPS)

        # xT per batch: [128 d, 8ch x 512 tok] per 512-token chunk (pre-chunked
        # contiguously on the host -> single-descriptor DMA), double buffered
        # so batch-1 chunks stream in as batch-0 finishes with them
        xT_sb = {}

        def load_xT(b, ck, queue=None):
            t_ = xtp.tile([128, 8 * 512], BF, tag=f"xtc{ck}", name=f"xt{b}_{ck}")
            (queue or nc.sync).dma_start(t_[:], io["xT"][b * 4 + ck])
            xT_sb[(b, ck)] = t_

        def xsl(b, tg, ch, c0, c1):  # x slice [128 d, c0:c1] of token group
            return xT_sb[(b, tg)][:, ch * 512 + c0: ch * 512 + c1]

        # ---- rms state ----
        # rms_tok: [128 part, NTT] f32; col b*16+tt = rsqrt(mean+eps) of token
        # tt*128+p of batch b  (exp scales + v scales read per-partition cols)
        rms_tok = singles.tile([128, NTT], F32)
        rms_bc = [None, None]
        rms_scr = dram.tile([T], F32, name="rms_scr")
        rms_scr1 = dram.tile([T], BF, name="rms_scr1")
        sums_row = singles.tile([1, T], F32)
        sqrt_row = singles.tile([1, 512], F32)
        # row-layout rms: reused for batch 1 after batch 0's broadcast is done
        rms_row = singles.tile([1, T], BF)
        ssq1 = singles.tile([128, 16], F32)    # batch-1 sumsq (DVE path)
        sq1t = singles.tile([128, 16], F32)
        rms16_bf = singles.tile([128, 16], BF)
        stt_dummy = singles.tile([128, 1], BF)
        rms_bc[0] = singles.tile([128, T], BF, tag="rmsbc0", name="rmsbc0")
        rms_bc[1] = singles.tile([128, T], BF, tag="rmsbc1", name="rmsbc1")

        # --- batch-0 sumsq on PE (ones-contraction over xT^2) ---
        def sumsq_chunk(b, ck):      # per 512-token chunk
            sq = work.tile([128, 8 * 512], BF, tag="sq", name="sq", bufs=1)
            nc.vector.tensor_mul(sq[:], xT_sb[(b, ck)][:], xT_sb[(b, ck)][:])
            ps = stg_ps.tile([1, 512], F32, tag="stg", name="sumsps",
                             padded_shape=[128, 512])
            for ch in range(8):
                nc.tensor.matmul(
                    ps[:], lhsT=ones128[:], rhs=sq[:, ch * 512:(ch + 1) * 512],
                    start=(ch == 0), stop=(ch == 7),
                )
            nc.vector.tensor_copy(sums_row[:, ck * 512:(ck + 1) * 512], ps[:])

        def bc_chunk(b, ck, row):
            # rank-1 broadcast chunk: rms_bc[b][d, ck*512+t] = row[ck*512+t]
            bc_ps = stg_ps.tile([128, 512], F32, tag="stg", name="bcps")
            nc.tensor.matmul(
                bc_ps[:], lhsT=ones_col[:],
                rhs=row[:, ck * 512:(ck + 1) * 512],
                start=True, stop=True,
            )
            nc.vector.tensor_copy(
                rms_bc[b][:, ck * 512:(ck + 1) * 512], bc_ps[:])

        def rms_rsqrt(ck):
            # batch-0 rsqrt per 512-token chunk (all four run at startup so
            # the ACT sqrt table is loaded exactly once before the exps)
            cs = slice(ck * 512, (ck + 1) * 512)
            nc.scalar.activation(
                sqrt_row[:], sums_row[:, cs],
                mybir.ActivationFunctionType.Sqrt,
                bias=eps_row[:], scale=1.0 / D,
            )
            nc.vector.reciprocal(sums_row[:, cs], sqrt_row[:])

        def rms_tail(ck):
            # bf16 cast + q-broadcast + DRAM bounce into [128 part, tt]
            # layout; bounces ride the scalar queue so they never sit behind
            # the big x loads on sync
            cs = slice(ck * 512, (ck + 1) * 512)
            nc.vector.tensor_copy(rms_row[:, cs], sums_row[:, cs])
            bc_chunk(0, ck, rms_row)
            nc.scalar.dma_start(rms_scr[cs], sums_row[:, cs])
            nc.scalar.dma_start(
                rms_tok[:, 4 * ck: 4 * ck + 4],
                rms_scr[cs].rearrange("(tt p) -> p tt", p=128),
            )

        # --- batch-1 sumsq on DVE from token-major x tiles ---
        # pairs of [128, D] tiles share the big "sq" work slot (idle after
        # the batch-0 startup squares)
        def load_xrms_pair(j):       # global tiles 16+j, 17+j (batch 1)
            t_ = work.tile([128, 2 * D], BF, tag="sq", name=f"xr{j}", bufs=1)
            nc.scalar.dma_start(
                t_.rearrange("p (k d) -> p k d", k=2),
                io["xrms"][16 + j: 18 + j].rearrange("k p d -> p k d"))
            return t_

        def sumsq_dve(j, t_, half):
            sl = t_[:, half * D:(half + 1) * D]
            nc.vector.scalar_tensor_tensor(
                stt_dummy.broadcast_to(sl.shape), sl, 1.0, sl,
                op0=AluOpType.mult, op1=AluOpType.mult,
                accum_out=ssq1[:, j:j + 1])

        def rms_finish1():
            nc.scalar.activation(
                sq1t[:], ssq1[:], mybir.ActivationFunctionType.Sqrt,
                bias=eps_col[:], scale=1.0 / D,
            )
            nc.vector.reciprocal(rms_tok[:, 16:32], sq1t[:])
            nc.vector.tensor_copy(rms16_bf[:], rms_tok[:, 16:32])
            # bounce [128, 16] -> [1, 2048] row layout for the q-rms broadcast
            nc.scalar.dma_start(
                rms_scr1.rearrange("(tt p) -> p tt", p=128), rms16_bf[:])
            nc.scalar.dma_start(rms_row[:], rms_scr1[:])
            for ck in range(4):
                bc_chunk(1, ck, rms_row)

        # ---- stage B: QKV + RoPE, per 512-token group ----
        qblk = singles.tile([128, BT], BF)
        kblk = singles.tile([128, BT], BF)
        v_aug = {}
        for b in range(B):
            for h in range(HPC):
                for kt in range(NKT):
                    v_aug[(b, h, kt)] = singles.tile(
                        [128, 65], BF, tag=f"va{h}_{kt}", bufs=2,
                        name=f"va{b}_{h}_{kt}")

        def qk_proj(b, tg, jt):      # one jt (0=q, 1=k) for one token group
            blk = qblk if jt == 0 else kblk
            tl = tg * 512            # batch-local col offset
            g = slice(b * T + tl, b * T + tl + 512)
            ps = stg_ps.tile([128, 512], F32, tag="stg", name="qkps")
            for ch in range(8):
                c0 = (jt * 8 + ch) * 128
                nc.tensor.matmul(
                    ps[:], lhsT=wq_sb[:, c0: c0 + 128],
                    rhs=xsl(b, tg, ch, 0, 512),
                    start=(ch == 0), stop=(ch == 7),
                )
            nc.vector.tensor_copy(blk[:, g], ps[:])

        def rope(b, tg, jt):         # in-place RoPE on q/k token group
            blk = qblk if jt == 0 else kblk
            tl = tg * 512
            g = slice(b * T + tl, b * T + tl + 512)
            cs = slice(tl, tl + 512)
            m1 = work.tile([128, 512], BF, tag="m1", name="m1")
            nc.vector.tensor_mul(m1[:], blk[:, g], cos_sb[:, cs])
            rot_ps = stg_ps.tile([128, 512], F32, tag="stg", name="rotps")
            nc.tensor.matmul(rot_ps[:], lhsT=ropep_sb[:], rhs=blk[:, g],
                             start=True, stop=True)
            m2 = work.tile([128, 512], BF, tag="m2", name="m2")
            nc.vector.tensor_mul(m2[:], rot_ps[:], sin_sb[:, cs])
            nc.vector.tensor_add(blk[:, g], m1[:], m2[:])

        def q_rms(b, tg):
            tl = tg * 512
            g = slice(b * T + tl, b * T + tl + 512)
            nc.vector.tensor_mul(qblk[:, g], qblk[:, g], rms_bc[b][:, tl: tl + 512])

        def v_proj_mm(b, tg):        # token-major V matmuls for one token group
            ps = stg_ps.tile([128, 512], F32, tag="stg", name="vfps")
            for ch in range(8):
                for tt in range(4):
                    nc.tensor.matmul(
                        ps[:, tt * 128:(tt + 1) * 128],
                        lhsT=xsl(b, tg, ch, tt * 128, (tt + 1) * 128),
                        rhs=wq_sb[:, (16 + ch) * 128: (17 + ch) * 128],
                        start=(ch == 0), stop=(ch == 7),
                    )
            vtmp = work.tile([128, 512], BF, tag="vtmp", name="vtmp")
            nc.vector.tensor_copy(vtmp[:], ps[:])
            return vtmp

        def v_scale(b, tg, vtmp):    # rms-scale V into v_aug tiles (gpsimd)
            for tt in range(4):
                kt = tg * 4 + tt
                for h in range(HPC):
                    va = v_aug[(b, h, kt)]
                    nc.gpsimd.tensor_scalar_mul(
                        va[:, 0:64], vtmp[:, tt * 128 + h * 64: tt * 128 + h * 64 + 64],
                        rms_tok[:, b * 16 + kt: b * 16 + kt + 1],
                    )
                    nc.gpsimd.memset(va[:, 64:65], 1.0)

        def v_proj(b, tg):
            v_scale(b, tg, v_proj_mm(b, tg))

        # ---- stage C/D state ----
        # one AllToAll per (batch, half): core c owns tokens
        # [half*1024 + c*128, +128) of each batch
        a2a_in = {(b, hf): dram.tile([T // 2, 128], BF, name=f"a2ain{b}{hf}")
                  for b in range(B) for hf in range(2)}
        a2a_out = {(b, hf): dram.tile([T // 2, 128], BF, name=f"a2aout{b}{hf}")
                   for b in range(B) for hf in range(2)}
        attn_sb = [singles.tile([128, T], BF, tag=f"attn{b}", name=f"attn{b}")
                   for b in range(B)]
        wo_sb = [None]
        xres_sb = [None]

        def load_stage_d():
            w = singles.tile([128, 8 * D], BF, tag="wo", name="wo")
            nc.scalar.dma_start(w[:], io["wo"][:])
            wo_sb[0] = w
            xr = singles.tile([128, 4 * D], F32, tag="xres", name="xres")
            nc.scalar.dma_start(xr[:], io["xres"][:])
            xres_sb[0] = xr

        def a2a_store(b, qg, h):     # one head's attn cols for one q-group
            hf, sub = qg // 2, qg % 2
            fs = slice(h * 64, (h + 1) * 64)
            nc.sync.dma_start(
                a2a_in[(b, hf)].rearrange("(qc p) f -> p qc f", p=128)[:, sub * 4:(sub + 1) * 4, fs],
                attn_sb[b].rearrange("p (qc f) -> p qc f", f=128)[:, qg * 4:(qg + 1) * 4, fs],
            )

        def a2a_go(b, hf):
            if with_collective:
                nc.gpsimd.collective_compute(
                    "AllToAll", mybir.AluOpType.bypass,
                    replica_groups=[list(range(N_CORES))],
                    ins=[a2a_in[(b, hf)].opt()], outs=[a2a_out[(b, hf)].opt()],
                )
            else:
                nc.sync.dma_start(a2a_out[(b, hf)][:], a2a_in[(b, hf)][:])

        attnT = {}

        def recv(b, hf):             # two transposed loads: [tok, f] -> [f, tok]
            # half-granular so out-proj mms on ch 0-3 can start while the
            # second half is still transposing
            # scalar queue: keeps the transposes' completion sems independent
            # of the sync queue's a2a stores/copies (no cross-coupling waits)
            t_ = epi_pool.tile([128, T // 2], BF, tag="attnT", name=f"attnT{b}{hf}")
            for half in range(2):
                ts = slice(half * 512, (half + 1) * 512)
                nc.scalar.dma_start(t_[:, ts], a2a_out[(b, hf)][ts, :],
                                    transpose=True)
            attnT[(b, hf)] = t_

        # out-proj for one (b, hf): 2 nh-halves x 2 ch-groups of 4 matmuls,
        # each half in its own [128, 512] PSUM bank from the given pool.
        def op_mms(b, hf, nh, ps, c0, c1):
            for ch in range(c0, c1):
                nc.tensor.matmul(
                    ps[:],
                    lhsT=attnT[(b, hf)][:, ch * 128:(ch + 1) * 128],
                    rhs=wo_sb[0][:, ch * D + nh * 512: ch * D + (nh + 1) * 512],
                    start=(ch == 0), stop=(ch == 7),
                )

        def op_finish(b, hf, nh, ps):
            ot = out_sb.tile([128, 512], F32, tag="ot", name="ot")
            nc.vector.tensor_add(
                ot[:], ps[:],
                xres_sb[0][:, (b * 2 + hf) * D + nh * 512:
                           (b * 2 + hf) * D + (nh + 1) * 512])
            r0 = (b * 2 + hf) * 128
            nc.sync.dma_start(
                io["y"][r0: r0 + 128, nh * 512:(nh + 1) * 512], ot[:])

        def outproj_fillers(bh_list, pool):
            """recv + 2 nh x 2 pieces per (b, hf); each piece ~0.9us of PE."""
            fl = []
            holder = {}
            for (b, hf) in bh_list:
                def f_recv(b=b, hf=hf):
                    recv(b, hf)
                fl.append(f_recv)
                for nh in range(2):
                    def f_a(b=b, hf=hf, nh=nh):
                        ps = pool.tile([128, 512], F32, tag="stg", name="opps")
                        holder[(b, hf, nh)] = ps
                        op_mms(b, hf, nh, ps, 0, 4)
                    def f_b(b=b, hf=hf, nh=nh):
                        ps = holder[(b, hf, nh)]
                        op_mms(b, hf, nh, ps, 4, 8)
                        op_finish(b, hf, nh, ps)
                    fl += [f_a, f_b]
            return fl

        # =================== emission schedule ===================
        # startup: only what gates qg0's early kt-steps runs before the
        # attention loop: qk/rope for tg0-1, sumsq chunks 0-1 with their
        # chunk-granular rms chains, q_rms(0,0).  Everything else is slotted
        # into specific kt-steps of the attention sweeps.
        wq_dma()
        load_xT(0, 0)
        trig_dma()
        load_xT(0, 2, nc.scalar)
        load_xT(0, 3, nc.scalar)
        load_xT(0, 1)
        qk_proj(0, 0, 0)
        sumsq_chunk(0, 0)
        qk_proj(0, 0, 1)
        rms_rsqrt(0)
        rms_tail(0)
        sumsq_chunk(0, 1)
        rope(0, 0, 0)
        rope(0, 0, 1)
        rms_rsqrt(1)
        rms_tail(1)
        sumsq_chunk(0, 2)
        qk_proj(0, 1, 1)
        rms_rsqrt(2)
        rms_tail(2)
        sumsq_chunk(0, 3)
        rope(0, 1, 1)
        q_rms(0, 0)
        rms_rsqrt(3)
        rms_tail(3)
        qk_proj(0, 1, 0)
        rope(0, 1, 0)

        # rest of b0 stage B goes into qg0's kt-steps at hand-placed slots.
        # deadlines: kblk(tg) before step 4*tg; rms_tok col kt (bounced DMA)
        # well before step kt; v_aug(tg) before step 4*tg + av_lag.
        vtmp_hold = [None]

        def vmm(b, tg):
            vtmp_hold[0] = v_proj_mm(b, tg)

        def vsc(b, tg):
            v_scale(b, tg, vtmp_hold[0])

        slots_b0 = {
            0: [lambda: qk_proj(0, 2, 1)],
            1: [lambda: rope(0, 2, 1)],
            2: [lambda: vmm(0, 0)],
            3: [lambda: vsc(0, 0)],
            4: [lambda: qk_proj(0, 3, 1)],
            5: [lambda: rope(0, 3, 1)],
            6: [lambda: vmm(0, 1)],
            7: [lambda: vsc(0, 1)],
            8: [lambda: qk_proj(0, 2, 0)],
            9: [lambda: rope(0, 2, 0)],
            10: [lambda: vmm(0, 2)],
            11: [lambda: vsc(0, 2)],
            12: [lambda: q_rms(0, 1), lambda: qk_proj(0, 3, 0)],
            13: [lambda: rope(0, 3, 0)],
            14: [lambda: vmm(0, 3)],
            15: [lambda: vsc(0, 3), lambda: q_rms(0, 2)],
        }

        # batch-1 stage B chopped into small bursts over qg1-3 of b0 attn.
        # xrms loads + DVE sumsq are interleaved early so rms_finish1 (ACT,
        # same table set) is ready before q_rms(1,*) / v_proj(1,*).
        xr_hold = [None]

        def xr_load(j):
            xr_hold[0] = load_xrms_pair(j)

        def xr_ssq(j):
            sumsq_dve(j, xr_hold[0], 0)
            sumsq_dve(j + 1, xr_hold[0], 1)

        # b0 sweep hosts: b1's K projections + rope, b1 V, b1 rms, and the
        # early q-side (tg0/tg1).  b1's late q-side (tg2/tg3) moves into
        # b1's own (ACT-bound, filler-starved) sweep.
        fill_b0 = [lambda: q_rms(0, 3), load_stage_d]
        for ck in range(4):
            fill_b0.append(lambda ck=ck: load_xT(1, ck))
        for j in range(0, 16, 2):
            fill_b0.append(lambda j=j: xr_load(j))
            fill_b0.append(lambda j=j: xr_ssq(j))
        for tg in range(4):
            fill_b0 += [
                lambda tg=tg: qk_proj(1, tg, 1),
                lambda tg=tg: rope(1, tg, 1),
            ]
        fill_b0 += [
            lambda: qk_proj(1, 0, 0), lambda: rope(1, 0, 0),
            lambda: qk_proj(1, 1, 0), lambda: rope(1, 1, 0),
        ]
        fill_b0.append(rms_finish1)
        fill_b0 += [lambda: q_rms(1, 0), lambda: q_rms(1, 1)]
        for tg in range(4):
            fill_b0.append(lambda tg=tg: v_proj(1, tg))

        # attention + interleave
        attn_ctx = ctx.enter_context(ExitStack())
        st_ps = attn_ctx.enter_context(tc.tile_pool(name="st_ps", bufs=2, space="PSUM"))
        av_ps = attn_ctx.enter_context(tc.tile_pool(name="av_ps", bufs=1, space="PSUM"))

        def attention_batch(b, fillers, first_step=4, av_lag=0, pre_slots=None,
                            post_cb=None):
            # spread fillers roughly evenly across the 64 kt-steps
            nsteps = NQG * NKT
            slots = dict(pre_slots or {})
            span = nsteps - first_step
            for i, f in enumerate(fillers):
                step = min(nsteps - 1,
                           first_step + (i * span) // max(len(fillers), 1))
                slots.setdefault(step, []).append(f)
            step = 0

            def av_mm(qg, kt, avs, exs):
                for h in range(HPC):
                    for qc in range(4):
                        nc.tensor.matmul(
                            avs[h][:, qc * 65:(qc + 1) * 65],
                            lhsT=exs[kt][:, h * QG + qc * 128: h * QG + (qc + 1) * 128],
                            rhs=v_aug[(b, h, kt)][:],
                            start=(kt == 0), stop=(kt == NKT - 1),
                        )

            for qg in range(NQG):
                q0 = b * T + qg * QG
                lag = av_lag if qg == 0 else 0
                exs = {}
                avs = [av_ps.tile([128, 260], F32, tag=f"av{h}", name=f"av{h}")
                       for h in range(HPC)]
                for kt in range(NKT):
                    st = st_ps.tile([128, 2 * QG], F32, tag="st", name="st")
                    for h in range(HPC):
                        o = h * 64
                        nc.tensor.matmul(
                            st[:, h * QG:(h + 1) * QG],
                            lhsT=kblk[o: o + 64, b * T + kt * 128: b * T + (kt + 1) * 128],
                            rhs=qblk[o: o + 64, q0: q0 + QG],
                            start=True, stop=True,
                        )
                    ex = exp_pool.tile([128, 2 * QG], BF, tag="ex", name="ex")
                    nc.scalar.activation(
                        ex[:], st[:], mybir.ActivationFunctionType.Exp,
                        scale=rms_tok[:, b * 16 + kt: b * 16 + kt + 1],
                    )
                    exs[kt] = ex
                    for f in slots.get(step, ()):
                        f()
                    if kt >= lag:
                        av_mm(qg, kt - lag, avs, exs)
                    step += 1
                for kt in range(NKT - lag, NKT):
                    av_mm(qg, kt, avs, exs)
                # epilogue: normalize into attn_sb (token-major).
                # PSUM->SBUF copy + strided reciprocal on DVE, scalar
                # multiplies on gpsimd (which cannot touch PSUM).
                for h in range(HPC):
                    avsb = epi_pool.tile([128, 260], BF, tag="avsb", name="avsb")
                    nc.vector.tensor_copy(avsb[:], avs[h][:])
                    rc = epi_pool.tile([128, 4], F32, tag="rc", name="rc")
                    nc.vector.reciprocal(
                        rc[:],
                        avsb.rearrange("p (qc c) -> p qc c", c=65)[:, :, 64:65])
                    for qc in range(4):
                        dst_c = (qg * 4 + qc) * 128 + h * 64
                        nc.gpsimd.tensor_scalar_mul(
                            attn_sb[b][:, dst_c: dst_c + 64],
                            avsb[:, qc * 65: qc * 65 + 64],
                            rc[:, qc: qc + 1],
                        )
                    a2a_store(b, qg, h)
                if qg == 1:
                    a2a_go(b, 0)
                elif qg == 3:
                    a2a_go(b, 1)
                if post_cb:
                    post_cb(qg)

        # batch-0 sweep: stage-B of b1 plus, once a2a(0,0) has landed
        # (fired after qg1 = step 32), the (0,0) out-projection through the
        # stg pool — pinned to explicit late slots so recv can never be
        # emitted before the collective.
        op00 = outproj_fillers([(0, 0)], stg_ps)
        for s, f in zip((50, 53, 55, 57, 59), op00):
            slots_b0.setdefault(s, []).append(f)
        attention_batch(0, fill_b0, first_step=16, av_lag=6,
                        pre_slots=slots_b0)

        # batch-1 sweep: (0,1) out-projection early, b1's late q-side at its
        # qg deadlines, recv(1,0) after its collective.  The (1,0) and (1,1)
        # out-projections are HELD for the tail so PE has work during the
        # final epilogue + a2a + transpose chain.
        op01 = outproj_fillers([(0, 1)], stg_ps)
        slots_b1 = {s: [f] for s, f in zip((2, 4, 6, 8, 10), op01)}
        slots_b1[18] = [lambda: qk_proj(1, 2, 0)]
        slots_b1[20] = [lambda: rope(1, 2, 0)]
        slots_b1[22] = [lambda: q_rms(1, 2)]
        slots_b1[34] = [lambda: qk_proj(1, 3, 0)]
        slots_b1[36] = [lambda: rope(1, 3, 0)]
        slots_b1[38] = [lambda: q_rms(1, 3)]
        op10 = outproj_fillers([(1, 0)], stg_ps)
        slots_b1[40] = [op10[0]]        # recv(1,0): a2a fired after qg1
        tail10 = op10[1:]
        attention_batch(1, [], first_step=2, pre_slots=slots_b1)
        # tail: (1,0) out-proj overlaps qg3's epilogue/a2a/recv-transpose
        for f in tail10:
            f()
        for f in outproj_fillers([(1, 1)], stg_ps):
            f()
        attn_ctx.close()
        stg_ctx.close()


def _prep_inputs(x, norm_w, w_qkv, w_out):
    """Host-side sharding. Returns list of per-core input dicts."""
    bf16 = ml_dtypes.bfloat16
    xf = np.ascontiguousarray(x.reshape(BT, D).astype(np.float32))
    # pre-chunked xT: [8 (b ck), 128 d-part, (8 c, 512 t)] so each 512-token
    # chunk is one contiguous DMA matching the SBUF tile layout
    xTf = xf.T.reshape(8, 128, 8, 512)              # [c, p, (b ck), t]
    xT = np.ascontiguousarray(
        xTf.transpose(2, 1, 0, 3).reshape(8, 128, 8 * 512)).astype(bf16)
    xrms = xf.reshape(NTT, 128, D).astype(bf16)

    w_eff = w_qkv.astype(np.float32) * norm_w.astype(np.float32)[None, :]
    scale = HD ** -0.5
    # rope tables (raw sin; rotate-half signs live in the permutation matrix)
    inv = 1.0 / (ROPE_BASE ** (np.arange(0, HD, 2, dtype=np.float32) / HD))
    t = np.arange(T, dtype=np.float32)
    fr = t[:, None] * inv[None, :]
    emb = np.concatenate([fr, fr], axis=-1)          # [T, 64]
    cos_b = np.concatenate([np.cos(emb).T] * 2, axis=0).astype(bf16)  # [128, T]
    sin_b = np.concatenate([np.sin(emb).T] * 2, axis=0).astype(bf16)

    # rotate-half permutation (acts on the partition/hd axis, per 64-block):
    # out[m] = -in[m+32] for m in [0,32), out[m] = in[m-32] for m in [32,64)
    p1 = np.zeros((64, 64), dtype=np.float32)
    for m in range(32):
        p1[m + 32, m] = -1.0     # lhsT[k, m] convention: out[m] = sum_k P[k,m] in[k]
        p1[m, m + 32] = 1.0
    ropep = np.zeros((128, 128), dtype=np.float32)
    ropep[0:64, 0:64] = p1
    ropep[64:128, 64:128] = p1
    ropep = ropep.astype(bf16)

    woT = np.ascontiguousarray(w_out.astype(np.float32).T)      # [1024 k, 1024 j]
    # SBUF layout [p, (c j)]
    wo = np.ascontiguousarray(
        woT.reshape(8, 128, D).transpose(1, 0, 2).reshape(128, 8 * D)
    ).astype(bf16)

    in_maps = []
    for c in range(N_CORES):
        h0, h1 = 2 * c, 2 * c + 1
        rows = []
        for part, s in ((0, scale), (1, 1.0), (2, 1.0)):  # q, k, v
            for h in (h0, h1):
                r = w_eff[part * D + h * HD: part * D + (h + 1) * HD] * s
                rows.append(r)
        wc = np.concatenate(rows, axis=0)            # [384, 1024]
        # SBUF layout [p, ((jt|v) c j)]: col (part*8 + c)*128 + j
        wqc = np.ascontiguousarray(
            wc.reshape(3, 128, 8, 128).transpose(3, 0, 2, 1)
            .reshape(128, 8 * JC)).astype(bf16)
        # token ownership: [b0h0, b0h1, b1h0, b1h1] x 128 tokens, laid out
        # [p, (k d)] to match the SBUF tile
        xres_c = np.ascontiguousarray(np.stack(
            [xf[b * T + hf * 1024 + c * 128: b * T + hf * 1024 + (c + 1) * 128]
             for b in range(B) for hf in range(2)],
            axis=1).reshape(128, 4 * D)).astype(np.float32)
        in_maps.append({
            "xT": xT, "xrms": xrms, "xres": xres_c,
            "wq": wqc, "wo": wo, "cosb": cos_b, "sinb": sin_b,
            "ropep": ropep,
        })
    return in_maps


_CACHE = {}


def _get_compiled():
    if "nc" not in _CACHE:
        nc = bacc.Bacc("TRN2", target_bir_lowering=False, debug=False,
                       num_devices=N_CORES)
        build_kernel(nc)
        nc.compile()
        _CACHE["nc"] = nc
    return _CACHE["nc"]


def kernel(x, norm_w, w_qkv, w_out):
    nc = _get_compiled()
    in_maps = _prep_inputs(np.asarray(x), np.asarray(norm_w),
                           np.asarray(w_qkv), np.asarray(w_out))
    res = run_bass_kernel_spmd(nc, in_maps, list(range(N_CORES)))
    y = np.empty((B, T, D), dtype=np.float32)
    for c in range(N_CORES):
        yc = res.results[c]["y"]
        for b in range(B):
            for hf in range(2):
                r0 = hf * 1024 + c * 128
                y[b, r0: r0 + 128] = yc[(b * 2 + hf) * 128:(b * 2 + hf + 1) * 128]
    return y
